# revision 13
# baseline (speedup 1.0000x reference)
"""Trainium2 Bass kernel for nn_BaseNetEmbedding (embedding-bag sum pool + 2x50 linear + relu).

Strategy (data-parallel, per the sharding hint):
  - Shard t [128, 2000] along batch across 8 cores -> 16 rows/core.
  - Replicate the [1M, 50] f32 embedding table (each core gathers locally).
  - Per core: 16*2000 = 32,000 indices laid out [128 partitions, 250 slots].
    Chunked indirect-DMA gather (SWDGE) pulls 200B rows into SBUF; DVE
    reduces each chunk over its slots while the next chunk's gather is in
    flight. A [128,16] selection matmul (values 1/128) folds the 8-partition
    group-sum and the /BATCH scale; the tiny 50->2 linear + bias + relu run
    on DVE. Output [16, 2] per core, concatenated on host.
"""

import sys

sys.path.insert(0, "/opt/trn_rl_repo")

import numpy as np

import concourse.bass as bass
import concourse.bacc as bacc
import concourse.mybir as mybir
import concourse.tile as tile
from concourse.bass_utils import run_bass_kernel_spmd

N_CORES = 8
BATCH = 128
SEQ = 2000
VOCAB = 1_000_000
EMB = 50
OUT = 2

P = 128                       # SBUF partitions
BPC = BATCH // N_CORES        # batch rows per core (16)
SLOTS = BPC * SEQ // P        # gather slots per partition (250)
ROWS_PER_PART = P // BPC      # partitions per batch row (8)
CHUNK = 50                    # slots gathered per indirect DMA
NCHUNK = SLOTS // CHUNK       # 5 chunks

FP = mybir.dt.float32


def build_program(vocab=VOCAB, slots=SLOTS, chunk=CHUNK, bpc=BPC):
    nchunk = slots // chunk
    # Bacc (vs raw Bass) so that compile() runs generate_event_semaphores,
    # which splits multi-wait instructions into the 1-wait-per-instruction
    # form the TRN2 walrus codegen requires.
    nc = bacc.Bacc()

    t32 = nc.dram_tensor("t32", [P, slots], mybir.dt.int32, kind="ExternalInput")
    emb = nc.dram_tensor("emb", [vocab, EMB], FP, kind="ExternalInput")
    # All small constants packed into one tensor (single DMA + single DVE
    # funnel copy keeps every consumer's sync fan-in at <=1 wait — this
    # walrus codegen path allows only one sync-wait per instruction):
    #   cols 0:bpc               selection matrix (1/BATCH at (p, p//ROWS_PER_PART))
    #   cols bpc:bpc+OUT*EMB     W replicated across partitions
    #   cols bpc+OUT*EMB:+OUT    bias replicated across partitions
    CW = bpc + OUT * EMB + OUT
    consts = nc.dram_tensor("consts", [P, CW], FP, kind="ExternalInput")
    res_d = nc.dram_tensor("res", [bpc, OUT], FP, kind="ExternalOutput")

    with tile.TileContext(nc) as tc:
        with (
            tc.tile_pool(name="const", bufs=1) as cpool,
            tc.tile_pool(name="gath", bufs=nchunk) as gpool,
            tc.tile_pool(name="red", bufs=2) as rpool,
            tc.tile_pool(name="work", bufs=1) as wpool,
            tc.tile_pool(name="psum", bufs=1, space="PSUM") as ppool,
        ):
            # Load indices via the Pool engine: the indirect gathers below are
            # also Pool-issued, so the one embedded wait on the first gather
            # plus same-queue FIFO ordering covers the dependency.
            idx_t = cpool.tile([P, slots], mybir.dt.int32)
            nc.gpsimd.dma_start(out=idx_t[:], in_=t32[:])
            consts_sb = cpool.tile([P, CW], FP)
            nc.sync.dma_start(out=consts_sb[:], in_=consts[:])
            consts_v = cpool.tile([P, CW], FP)
            nc.vector.tensor_copy(out=consts_v[:], in_=consts_sb[:])
            sel_t = consts_v[:, 0:bpc]
            w_t = consts_v[:bpc, bpc : bpc + OUT * EMB]
            b_t = consts_v[:bpc, bpc + OUT * EMB : CW]

            partial = wpool.tile([P, EMB], FP)
            for j in range(nchunk):
                g = gpool.tile([P, chunk * EMB], FP, tag="g")
                # The TRN2 indirect-DMA ucode consumes exactly one index per
                # partition per instruction (verified on HW: a [P, K] offset
                # AP gathers only idx[p, 0] and then K*E contiguous elements).
                # So issue one [P,1] gather per slot.
                for s in range(chunk):
                    nc.gpsimd.indirect_dma_start(
                        out=g[:, s * EMB : (s + 1) * EMB],
                        out_offset=None,
                        in_=emb[:],
                        in_offset=bass.IndirectOffsetOnAxis(
                            ap=idx_t[:, j * chunk + s : j * chunk + s + 1], axis=0
                        ),
                    )
                red = rpool.tile([P, EMB], FP, tag="red")
                nc.vector.reduce_sum(
                    out=red[:],
                    in_=g[:].rearrange("p (s e) -> p e s", s=chunk, e=EMB),
                    axis=mybir.AxisListType.X,
                )
                if j == 0:
                    nc.vector.tensor_copy(out=partial[:], in_=red[:])
                else:
                    nc.vector.tensor_add(out=partial[:], in0=partial[:], in1=red[:])

            pooled_ps = ppool.tile([bpc, EMB], FP, space="PSUM")
            nc.tensor.matmul(
                out=pooled_ps[:], lhsT=sel_t, rhs=partial[:], start=True, stop=True
            )
            pooled = wpool.tile([bpc, EMB], FP)
            nc.vector.tensor_copy(out=pooled[:], in_=pooled_ps[:])

            res_t = wpool.tile([bpc, OUT], FP)
            tmp = wpool.tile([bpc, EMB], FP)
            for o in range(OUT):
                nc.vector.tensor_mul(
                    out=tmp[:],
                    in0=pooled[:],
                    in1=consts_v[:bpc, bpc + o * EMB : bpc + (o + 1) * EMB],
                )
                nc.vector.reduce_sum(
                    out=res_t[:, o : o + 1], in_=tmp[:], axis=mybir.AxisListType.X
                )
            nc.vector.tensor_add(out=res_t[:], in0=res_t[:], in1=b_t)
            nc.vector.tensor_relu(out=res_t[:], in_=res_t[:])
            nc.sync.dma_start(out=res_d[:], in_=res_t[:])

    nc.compile()
    return nc


# ---------------- fast path: banked dma_gather ----------------------------
# The TRN2 SWDGE has a purpose-built gather instruction (InstDMAGatherAnt)
# that takes a whole int16 index list per instruction (0.34ns/descriptor
# generation vs ~1us per 128-row generic indirect DMA). Constraints: int16
# indices (so the 1M-row table is processed in 32768-row banks), 256B-
# aligned rows (table padded 50 -> 64 floats on host), gathered element i
# lands at dest[i%128, i//128, :]. Batch-row attribution and the /BATCH
# scale are folded into per-slot-column selection matmuls whose weights are
# built host-side; padded slots gather bank-row 0 and carry weight 0.

BANK = 32768
NBANK = (VOCAB + BANK - 1) // BANK  # 31
CAP = 1280                          # padded index count per bank (10 * 128)
COLS = CAP // 128                   # dest columns per bank
SLOTCOLS = NBANK * COLS             # 310 slot columns total
EPAD = 64                           # table row padded to 256B


def build_program_fast(nbank=NBANK, cap=CAP, vocab=VOCAB, bpc=BPC):
    cols = cap // 128
    slotcols = nbank * cols
    nc = bacc.Bacc()

    idx16 = nc.dram_tensor(
        "idx16", [P, nbank * (cap // 16)], mybir.dt.int16, kind="ExternalInput"
    )
    tabp = nc.dram_tensor("tabp", [vocab, EPAD], FP, kind="ExternalInput")
    selw = nc.dram_tensor("selw", [P, slotcols * bpc], FP, kind="ExternalInput")
    CW = bpc + OUT * EMB + OUT
    consts = nc.dram_tensor("consts", [P, CW], FP, kind="ExternalInput")
    res_d = nc.dram_tensor("res", [bpc, OUT], FP, kind="ExternalOutput")

    with tile.TileContext(nc) as tc:
        with (
            tc.tile_pool(name="const", bufs=1) as cpool,
            tc.tile_pool(name="work", bufs=1) as wpool,
            tc.tile_pool(name="psum", bufs=1, space="PSUM") as ppool,
        ):
            idx_t = cpool.tile([P, nbank * (cap // 16)], mybir.dt.int16)
            nc.gpsimd.dma_start(out=idx_t[:], in_=idx16[:])
            selw_t = cpool.tile([P, slotcols * bpc], FP)
            nc.sync.dma_start(out=selw_t[:], in_=selw[:])
            consts_sb = cpool.tile([P, CW], FP)
            nc.sync.dma_start(out=consts_sb[:], in_=consts[:])
            consts_v = cpool.tile([P, CW], FP)
            nc.vector.tensor_copy(out=consts_v[:], in_=consts_sb[:])
            b_t = consts_v[:bpc, bpc + OUT * EMB : CW]

            dest = cpool.tile([P, slotcols * EPAD], FP)
            dest3 = dest[:].rearrange("p (c e) -> p c e", c=slotcols, e=EPAD)
            for k in range(nbank):
                rows = min(BANK, vocab - k * BANK)
                nc.gpsimd.dma_gather(
                    dest3[:, k * cols : (k + 1) * cols, :],
                    tabp[k * BANK : k * BANK + rows, :],
                    idx_t[:, k * (cap // 16) : (k + 1) * (cap // 16)],
                    cap,
                    cap,
                    EPAD,
                    elem_step=EPAD,
                )

            pooled_ps = ppool.tile([bpc, EMB], FP, space="PSUM")
            for s in range(slotcols):
                nc.tensor.matmul(
                    out=pooled_ps[:],
                    lhsT=selw_t[:, s * bpc : (s + 1) * bpc],
                    rhs=dest3[:, s, 0:EMB],
                    start=(s == 0),
                    stop=(s == slotcols - 1),
                )
            pooled = wpool.tile([bpc, EMB], FP)
            nc.vector.tensor_copy(out=pooled[:], in_=pooled_ps[:])

            res_t = wpool.tile([bpc, OUT], FP)
            tmp = wpool.tile([bpc, EMB], FP)
            for o in range(OUT):
                nc.vector.tensor_mul(
                    out=tmp[:],
                    in0=pooled[:],
                    in1=consts_v[:bpc, bpc + o * EMB : bpc + (o + 1) * EMB],
                )
                nc.vector.reduce_sum(
                    out=res_t[:, o : o + 1], in_=tmp[:], axis=mybir.AxisListType.X
                )
            nc.vector.tensor_add(out=res_t[:], in0=res_t[:], in1=b_t)
            nc.vector.tensor_relu(out=res_t[:], in_=res_t[:])
            nc.sync.dma_start(out=res_d[:], in_=res_t[:])

    nc.compile()
    return nc


def make_in_maps_fast(t, embeddings, W, b, nbank=NBANK, cap=CAP, vocab=VOCAB, bpc=BPC):
    cols = cap // 128
    slotcols = nbank * cols
    t32 = np.ascontiguousarray(t.astype(np.int64)).reshape(BATCH, SEQ)
    W = np.asarray(W, dtype=np.float32)
    b = np.asarray(b, dtype=np.float32)

    tabp = np.zeros((vocab, EPAD), dtype=np.float32)
    tabp[:, :EMB] = np.asarray(embeddings, dtype=np.float32)

    CW = bpc + OUT * EMB + OUT
    consts = np.zeros((P, CW), dtype=np.float32)
    consts[:, bpc : bpc + OUT * EMB] = W.reshape(1, OUT * EMB)
    consts[:, bpc + OUT * EMB : CW] = b.reshape(1, OUT)

    in_maps = []
    for c in range(N_CORES):
        flat = t32[c * bpc : (c + 1) * bpc].reshape(-1)  # [bpc*SEQ]
        rows = np.arange(flat.size) // SEQ               # batch row within core
        bank = flat // BANK
        counts = np.bincount(bank, minlength=nbank)
        if counts.max() > cap:
            raise OverflowError(f"bank count {counts.max()} exceeds cap {cap}")
        order = np.argsort(bank, kind="stable")
        idx_arr = np.zeros((16, nbank * (cap // 16)), dtype=np.int16)
        selw_arr = np.zeros((P, slotcols * bpc), dtype=np.float32)
        pos = 0
        for k in range(nbank):
            n_k = counts[k]
            sel_idx = order[pos : pos + n_k]
            pos += n_k
            locs = (flat[sel_idx] - k * BANK).astype(np.int16)
            brows = rows[sel_idx]
            i = np.arange(n_k)
            idx_arr[i % 16, k * (cap // 16) + i // 16] = locs
            # padded entries (i >= n_k) keep index 0 and weight 0
            pcol = (k * cols + i // 128) * bpc + brows
            selw_arr[i % 128, pcol] = 1.0 / BATCH
        in_maps.append(
            {
                "idx16": np.tile(idx_arr, (8, 1)),
                "tabp": tabp,
                "selw": selw_arr,
                "consts": consts,
            }
        )
    return in_maps


def make_in_maps(t, embeddings, W, b, vocab=VOCAB, slots=SLOTS, bpc=BPC):
    t32 = np.ascontiguousarray(t.astype(np.int32))
    emb = np.ascontiguousarray(embeddings.astype(np.float32, copy=False))
    W = np.asarray(W, dtype=np.float32)
    b = np.asarray(b, dtype=np.float32)

    CW = bpc + OUT * EMB + OUT
    consts = np.zeros((P, CW), dtype=np.float32)
    consts[np.arange(P), np.arange(P) // ROWS_PER_PART] = 1.0 / BATCH
    consts[:, bpc : bpc + OUT * EMB] = W.reshape(1, OUT * EMB)
    consts[:, bpc + OUT * EMB : CW] = b.reshape(1, OUT)

    in_maps = []
    for c in range(N_CORES):
        t_c = t32[c * bpc : (c + 1) * bpc].reshape(P, slots)
        in_maps.append({"t32": t_c, "emb": emb, "consts": consts})
    return in_maps


_PROGRAM = None


def kernel(t, embeddings, W, b):
    global _PROGRAM
    if _PROGRAM is None:
        _PROGRAM = build_program()
    in_maps = make_in_maps(t, embeddings, W, b)
    results = run_bass_kernel_spmd(_PROGRAM, in_maps, core_ids=list(range(N_CORES)))
    out = np.concatenate([results.results[c]["res"] for c in range(N_CORES)], axis=0)
    return out.astype(np.float32)


if __name__ == "__main__":
    rng = np.random.default_rng(0)
    t = rng.integers(0, VOCAB, size=(BATCH, SEQ)).astype(np.int64)
    embeddings = (rng.standard_normal((VOCAB, EMB)) * 0.02).astype(np.float32)
    W = rng.standard_normal((OUT, EMB)).astype(np.float32) * 0.1
    b = np.zeros((OUT,), dtype=np.float32)
    got = kernel(t, embeddings, W, b)
    mask = np.ones_like(t, dtype=np.float32)
    pooled = embeddings[t].sum(axis=1) / BATCH
    want = np.maximum(pooled @ W.T + b, 0.0)
    err = np.abs(got - want).max() / (np.abs(want).max() + 1e-12)
    print("max rel err:", err)
